# revision 4
# baseline (speedup 1.0000x reference)
"""Trainium2 Bass kernel for CrossMerge3D.

Input ys: [B=2, S=12, C=96, 32, 32, 32] f32. For each (b, c):
  out = (mA + perm_j(mB) + perm_k(mC)) / 12
where, with the 12 scans split into 3 groups of 4, each group combines as
  m_g = s0 + s1 + flip(s2 + s3)   (flip over the flattened 32^3 volume)
and group B's volume is stored as (j,k,i), group C's as (k,i,j).

Sharding: 8 cores = batch (2) x channel quarters (4) -> 24 channels/core.
No cross-core communication.

Per-core layout: 4 channels x 32 leading-spatial -> 128 SBUF partitions,
1024-wide free dim, 6 macro tiles. One 2 MiB quad DMA per scan group.
Pair sums on DVE cast to bf16 (tolerance is 2e-2; bf16 keeps ~4e-3).
flip = free-dim reversal (pair-sum read APs) + partition-block reversal,
the latter fused with the group combine and the global 1/12 scale as
accumulating bf16 matmuls against 1/12-scaled stationaries (wJ = block
anti-diagonal, wI = identity). C's pair sums write with a permuted out-AP
so its whole downstream path is contiguous; B's leftover (k,j)->(j,k)
permute folds into the final DVE add's read AP. The final add reads
PSUM + bf16 and writes the f32 output tile directly - no scale op.
"""

import numpy as np

_B, _S, _C, _D = 2, 12, 96, 32
_NCORE = 8
_CL = _C // 4          # 24 channels per core
_G = _CL // 4          # 6 macro tiles of 4 channels (128 partitions)
_F = _D * _D           # 1024

_nc = None


def _build_program():
    from concourse import bacc, tile, mybir

    f32 = mybir.dt.float32
    bf16 = mybir.dt.bfloat16
    nc = bacc.Bacc(
        "TRN2", target_bir_lowering=False, debug=False, num_devices=_NCORE
    )
    ys = nc.dram_tensor("ys", [_S, _CL, _D, _D, _D], f32, kind="ExternalInput")
    out = nc.dram_tensor("out", [_CL, _D, _D, _D], f32, kind="ExternalOutput")
    ysa = ys.ap()
    outa = out.ap()

    with tile.TileContext(nc) as tc:
        with (
            tc.tile_pool(name="const", bufs=1) as cst,
            tc.tile_pool(name="io", bufs=2) as iop,
            tc.tile_pool(name="tmp", bufs=2) as tmp,
            tc.tile_pool(name="ps", bufs=2, space="PSUM") as ps,
        ):
            scale = 1.0 / 12.0
            # stationaries: 32-block anti-diagonal (x 1/12), identity
            # (x 1/12), and an unscaled identity for the pre-scaled tCt.
            wJ = cst.tile([128, 128], bf16, tag="wJ", name="wJ")
            nc.gpsimd.memset(wJ[:], scale)
            for b in range(4):
                nc.gpsimd.affine_select(
                    out=wJ[32 * b:32 * b + 32, :],
                    in_=wJ[32 * b:32 * b + 32, :],
                    compare_op=mybir.AluOpType.is_equal, fill=0.0,
                    base=-(32 * b + 31), pattern=[[1, 128]],
                    channel_multiplier=1,
                )
            wI = cst.tile([128, 128], bf16, tag="wI", name="wI")
            nc.gpsimd.memset(wI[:], scale)
            nc.gpsimd.affine_select(
                out=wI[:], in_=wI[:],
                compare_op=mybir.AluOpType.is_equal, fill=0.0,
                base=0, pattern=[[1, 128]], channel_multiplier=-1,
            )
            wI1 = cst.tile([128, 128], bf16, tag="wI1", name="wI1")
            nc.gpsimd.memset(wI1[:], 1.0)
            nc.gpsimd.affine_select(
                out=wI1[:], in_=wI1[:],
                compare_op=mybir.AluOpType.is_equal, fill=0.0,
                base=0, pattern=[[1, 128]], channel_multiplier=-1,
            )

            for g in range(_G):
                cs = slice(4 * g, 4 * (g + 1))

                def load_quad(s, tag, eng):
                    t = iop.tile([128, 4 * _F], f32, tag=tag, name=tag)
                    src = ysa[s:s + 4, cs].rearrange(
                        "s c i j k -> (c i) s (j k)"
                    )
                    dst = t[:].rearrange("p (s f) -> p s f", s=4)
                    eng.dma_start(out=dst, in_=src)
                    return t

                qB = load_quad(4, "qB", nc.sync)
                qC = load_quad(8, "qC", nc.scalar)
                qA = load_quad(0, "qA", nc.sync if g % 2 else nc.scalar)

                def pair_sums(q, tag, permute_out):
                    rs = tmp.tile([128, _F], bf16, tag="rs" + tag,
                                  name="rs" + tag)
                    fs = tmp.tile([128, _F], bf16, tag="fs" + tag,
                                  name="fs" + tag)
                    if permute_out:
                        # write (i,j) -> (j,i): stream (i outer, j inner),
                        # address = j*32 + i; flat reversal == reversing
                        # both free dims of the 3-D view
                        ro = rs[:].rearrange("p (j i) -> p i j", j=_D)
                        fo = fs[:].rearrange("p (j i) -> p i j", j=_D)
                        r3 = lambda a: a.rearrange("p (i j) -> p i j", i=_D)
                        rev = lambda a: r3(a)[:, ::-1, ::-1]
                    else:
                        ro, fo = rs[:], fs[:]
                        r3 = lambda a: a
                        rev = lambda a: a[:, ::-1]
                    nc.vector.tensor_add(
                        ro, rev(q[:, 2 * _F:3 * _F]),
                        rev(q[:, 3 * _F:4 * _F]))
                    nc.vector.tensor_add(
                        fo, r3(q[:, 0:_F]), r3(q[:, _F:2 * _F]))
                    return rs, fs

                def combine(rs, fs, name, first=None, last=None, bufs=2):
                    # psum = wJ/12 @ rs + wI/12 @ fs (+ optional pre/post
                    # accumulation members supplied by the caller).
                    # B and C share a tag so PSUM fits in 8 banks.
                    tag = "psA" if name == "A" else "psBC"
                    pf = ps.tile([128, _F], f32, tag=tag, name=name,
                                 bufs=bufs)
                    for n0 in (0, _F // 2):
                        h = slice(n0, n0 + _F // 2)
                        st = first is None
                        if first is not None:
                            first(pf, h)
                        nc.tensor.matmul(pf[:, h], wJ[:], rs[:][:, h],
                                         start=st, stop=False)
                        nc.tensor.matmul(pf[:, h], wI[:], fs[:][:, h],
                                         start=False, stop=last is None)
                        if last is not None:
                            last(pf, h)
                    return pf

                # B: needs j<->i 32x32 block transpose; leftover (k,j)
                # free permute is folded into the final add's read AP.
                rsB, fsB = pair_sums(qB, "B", False)
                psB = combine(rsB, fsB, "B")
                tB = tmp.tile([128, _F], bf16, tag="tB", name="tB")
                nc.scalar.copy(tB[:], psB[:])
                tBt = tmp.tile([128, _F], bf16, tag="tBt", name="tBt")
                nc.vector.transpose(tBt[:], tB[:])

                # C: pair sums wrote (j,i)-permuted, so copy + transpose
                # are contiguous and tCt is already in (j,k) layout.
                rsC, fsC = pair_sums(qC, "C", True)
                psC = combine(rsC, fsC, "C")
                tC = tmp.tile([128, _F], bf16, tag="tC", name="tC")
                nc.scalar.copy(tC[:], psC[:])
                tCt = tmp.tile([128, _F], bf16, tag="tCt", name="tCt")
                nc.vector.transpose(tCt[:], tC[:])

                # A accumulates C's contribution (pre-scaled, via the
                # unscaled identity) plus its own scans.
                rsA, fsA = pair_sums(qA, "A", False)

                def acc_tCt(pf, h):
                    nc.tensor.matmul(pf[:, h], wI1[:], tCt[:][:, h],
                                     start=True, stop=False)

                psA = combine(rsA, fsA, "A", first=acc_tCt)

                # out = psA + tBt read as (j,k); f32 write, no scale op.
                o = tmp.tile([128, _F], f32, tag="o", name="o")
                nc.vector.tensor_add(
                    o[:].rearrange("p (j k) -> p j k", j=_D),
                    psA[:].rearrange("p (j k) -> p j k", j=_D),
                    tBt[:].rearrange("p (k j) -> p j k", k=_D),
                )
                (nc.scalar if g % 2 else nc.sync).dma_start(
                    out=outa[cs].rearrange("c i j k -> (c i) (j k)"), in_=o[:]
                )

    nc.compile()
    return nc


def kernel(ys):
    global _nc
    ys = np.ascontiguousarray(ys, dtype=np.float32)
    assert ys.shape == (_B, _S, _C, _D, _D, _D), ys.shape

    if _nc is None:
        _nc = _build_program()

    from concourse.bass_utils import run_bass_kernel_spmd

    in_maps = []
    for r in range(_NCORE):
        b, q = divmod(r, 4)
        shard = np.ascontiguousarray(ys[b, :, q * _CL:(q + 1) * _CL])
        in_maps.append({"ys": shard})

    res = run_bass_kernel_spmd(_nc, in_maps, list(range(_NCORE)))

    out = np.empty((_B, _C, _D, _D, _D), np.float32)
    for r in range(_NCORE):
        b, q = divmod(r, 4)
        out[b, q * _CL:(q + 1) * _CL] = res.results[r]["out"]

    if res.exec_time_ns is not None:
        print(f"HW exec time: {res.exec_time_ns} ns")
    return out
